# revision 10
# baseline (speedup 1.0000x reference)
"""Trainium2 Bass kernel for nn_Attention_15539191677265.

Single-head-dim attention block:
    qkv = w_qkv @ x ; per-head scaled dot-product attention over w=2048;
    out = w_out @ attn_out + b_out

Sharding: pure data-parallel over batch (b=8 -> 8 NeuronCores, one batch
element per core). Weights are replicated. No collectives.

Per-core algorithm (transposed-softmax scheme, all matmuls bf16):
  1. q,k = wqkvT.T @ x           ([c,o] stationary; q pre-scaled on host)
  2. vT  = x.T @ wvT             (v produced directly transposed [j, d])
  3. per head: sim^T[j,i] strips -> exp on ScalarE (no max subtraction:
     scores are ~N(0,1), exp cannot overflow in fp32/bf16 range)
  4. AV: out^T[d,i] = vT.T @ exp_strip, with a ones-column appended to vT
     so row 64 of the psum accumulates the softmax normalizer for free
  5. normalize: reciprocal(norm row) -> partition_broadcast -> multiply
  6. proj: out = woutT.T @ attn_out (per-head K=64 chunks) + bias
"""

import sys

if "/opt/trn_rl_repo" not in sys.path:
    sys.path.insert(0, "/opt/trn_rl_repo")

import numpy as np
import ml_dtypes

import concourse.bass as bass
import concourse.mybir as mybir
import concourse.tile as tile
from concourse import bacc
from concourse.bass_utils import run_bass_kernel_spmd

BF16 = mybir.dt.bfloat16
F32 = mybir.dt.float32
EXP = mybir.ActivationFunctionType.Exp

B, DIM, W = 8, 256, 2048
HEADS, DH = 8, 64
HID = HEADS * DH  # 512
SCALE = DH ** (-0.5)
N_CORES = 8

NJT = W // 128  # 16 j-tiles per head
NCT = DIM // 128  # 2 contraction chunks over channels


def build_kernel():
    nc = bacc.Bacc(None, target_bir_lowering=False)

    x_d = nc.dram_tensor("x", [DIM, W], BF16, kind="ExternalInput")
    wqkvT_d = nc.dram_tensor("wqkvT", [DIM, 3 * HID], BF16, kind="ExternalInput")
    woutT_d = nc.dram_tensor("woutT", [128, 4, DIM], BF16, kind="ExternalInput")
    bias_d = nc.dram_tensor("bias", [128, DIM // 128], F32, kind="ExternalInput")
    out_d = nc.dram_tensor("out", [DIM, W], F32, kind="ExternalOutput")

    with tile.TileContext(nc) as tc:
        with tc.tile_pool(name="pers", bufs=1) as pers:
            x_sb = pers.tile([128, NCT, W], BF16)
            wq_sb = pers.tile([128, NCT, 3 * HID], BF16)
            wo_sb = pers.tile([128, 4, DIM], BF16)
            bias_sb = pers.tile([128, DIM // 128], F32)
            q_sb = pers.tile([128, 4, W], BF16)
            k_sb = pers.tile([128, 4, W], BF16)
            vt_sb = pers.tile([128, NJT, HEADS, DH + 1], BF16)
            attout_sb = pers.tile([128, 4, W], BF16)
            out_sb = pers.tile([128, NCT, W], F32)

            xr = x_d[:].rearrange("(ct p) w -> p ct w", p=128)
            for ct in range(NCT):
                nc.sync.dma_start(out=x_sb[:, ct, :], in_=xr[:, ct, :])
            nc.sync.dma_start(
                out=wq_sb[:], in_=wqkvT_d[:].rearrange("(ct p) o -> p ct o", p=128)
            )
            nc.sync.dma_start(out=wo_sb[:], in_=woutT_d[:])
            nc.sync.dma_start(out=bias_sb[:], in_=bias_d[:])

            # ones column of vT (col 64 of each head block)
            nc.vector.memset(vt_sb[:, :, :, DH : DH + 1], 1.0)

            # ---- phase 1: q, k projections -> [128, 4, W] bf16 each ----
            with tc.tile_pool(name="qkv_ps", bufs=2, space="PSUM") as qkv_ps:
                for ot in range(8):  # o-tiles 0..3 = q, 4..7 = k
                    ps = qkv_ps.tile([128, W], F32)
                    for ct in range(NCT):
                        for ns in range(4):
                            nc.tensor.matmul(
                                ps[:, ns * 512 : (ns + 1) * 512],
                                lhsT=wq_sb[:, ct, ot * 128 : (ot + 1) * 128],
                                rhs=x_sb[:, ct, ns * 512 : (ns + 1) * 512],
                                start=(ct == 0),
                                stop=(ct == NCT - 1),
                            )
                    dst = q_sb if ot < 4 else k_sb
                    nc.vector.tensor_copy(out=dst[:, ot % 4, :], in_=ps[:])

            # ---- phase 2: vT[j, hd] = x.T @ wvT  (+ones col kept) ----
            with tc.tile_pool(name="vt_ps", bufs=2, space="PSUM") as vt_ps:
                for jt in range(NJT):
                    ps = vt_ps.tile([128, HID], F32)
                    for ct in range(NCT):
                        nc.tensor.matmul(
                            ps[:],
                            lhsT=x_sb[:, ct, jt * 128 : (jt + 1) * 128],
                            rhs=wq_sb[:, ct, 2 * HID : 3 * HID],
                            start=(ct == 0),
                            stop=(ct == NCT - 1),
                        )
                    nc.vector.tensor_copy(
                        out=vt_sb[:, jt, :, 0:DH],
                        in_=ps[:].rearrange("p (h d) -> p h d", h=HEADS),
                    )

            # ---- phase 3: attention per head ----
            with (
                tc.tile_pool(name="strip_ps", bufs=2, space="PSUM") as strip_ps,
                tc.tile_pool(name="av_ps", bufs=2, space="PSUM") as av_ps,
                tc.tile_pool(name="exp_sb", bufs=4) as exp_pool,
                tc.tile_pool(name="norm_sb", bufs=2) as norm_pool,
            ):
                for h in range(HEADS):
                    kt, koff = h // 2, (h % 2) * 64
                    avs = [
                        av_ps.tile([DH + 1, 1024], F32, name=f"av_{h}_{ih}", tag="av")
                        for ih in range(2)
                    ]
                    for jt in range(NJT):
                        # 4 sim matmuls sharing one k-tile LDWEIGHTS, then
                        # 4 AV matmuls sharing one vT LDWEIGHTS
                        strips = []
                        for ih in range(2):
                            io = ih * 1024
                            strip = strip_ps.tile(
                                [128, 1024], F32, name=f"st_{ih}", tag="st"
                            )
                            strips.append(strip)
                            for ns in range(2):
                                nc.tensor.matmul(
                                    strip[:, ns * 512 : (ns + 1) * 512],
                                    lhsT=k_sb[
                                        koff : koff + DH, kt, jt * 128 : (jt + 1) * 128
                                    ],
                                    rhs=q_sb[
                                        koff : koff + DH,
                                        kt,
                                        io + ns * 512 : io + (ns + 1) * 512,
                                    ],
                                    start=True,
                                    stop=True,
                                )
                        ess = []
                        for ih in range(2):
                            es = exp_pool.tile(
                                [128, 1024], BF16, name=f"es_{ih}", tag="es"
                            )
                            ess.append(es)
                            nc.scalar.activation(
                                out=es[:], in_=strips[ih][:], func=EXP
                            )
                        for ih in range(2):
                            for ns in range(2):
                                nc.tensor.matmul(
                                    avs[ih][:, ns * 512 : (ns + 1) * 512],
                                    lhsT=vt_sb[:, jt, h, :],
                                    rhs=ess[ih][:, ns * 512 : (ns + 1) * 512],
                                    start=(jt == 0),
                                    stop=(jt == NJT - 1),
                                )
                    for ih in range(2):
                        io = ih * 1024
                        av = avs[ih]
                        rec = norm_pool.tile([DH + 1, 1024], F32, tag="rec")
                        rec0 = norm_pool.tile([1, 1024], F32, tag="rec0")
                        bc = norm_pool.tile([DH, 1024], F32, tag="bc")
                        nc.vector.reciprocal(
                            out=rec[DH : DH + 1, :], in_=av[DH : DH + 1, :]
                        )
                        # partition_broadcast only reads partition 0 -> move row
                        nc.sync.dma_start(out=rec0[:], in_=rec[DH : DH + 1, :])
                        nc.gpsimd.partition_broadcast(
                            bc[:], rec0[0:1, :], channels=DH
                        )
                        if h % 2 == 0:
                            nc.vector.tensor_mul(
                                out=attout_sb[0:DH, h // 2, io : io + 1024],
                                in0=av[0:DH, :],
                                in1=bc[:],
                            )
                        else:
                            # odd heads land on partitions 64..127: DVE cannot
                            # shift partitions, so write via a bounce + DMA
                            atmp = norm_pool.tile([DH, 1024], BF16, tag="atmp")
                            nc.vector.tensor_mul(
                                out=atmp[:], in0=av[0:DH, :], in1=bc[:]
                            )
                            nc.sync.dma_start(
                                out=attout_sb[DH:128, h // 2, io : io + 1024],
                                in_=atmp[:],
                            )

            # ---- phase 4: output projection + bias (K=128 head pairs) ----
            outr = out_d[:].rearrange("(ct p) w -> p ct w", p=128)
            with tc.tile_pool(name="proj_ps", bufs=2, space="PSUM") as proj_ps:
                for ot in range(NCT):
                    for wh in range(2):
                        wo = wh * 1024
                        ps = proj_ps.tile([128, 1024], F32)
                        for kc in range(4):
                            for ns in range(2):
                                nc.tensor.matmul(
                                    ps[:, ns * 512 : (ns + 1) * 512],
                                    lhsT=wo_sb[:, kc, ot * 128 : (ot + 1) * 128],
                                    rhs=attout_sb[
                                        :, kc, wo + ns * 512 : wo + (ns + 1) * 512
                                    ],
                                    start=(kc == 0),
                                    stop=(kc == 3),
                                )
                        nc.vector.tensor_scalar_add(
                            out=out_sb[:, ot, wo : wo + 1024],
                            in0=ps[:],
                            scalar1=bias_sb[:, ot : ot + 1],
                        )
                        nc.sync.dma_start(
                            out=outr[:, ot, wo : wo + 1024],
                            in_=out_sb[:, ot, wo : wo + 1024],
                        )

    nc.compile()
    return nc


_NC_CACHE = None


def _get_nc():
    global _NC_CACHE
    if _NC_CACHE is None:
        _NC_CACHE = build_kernel()
    return _NC_CACHE


def make_in_maps(x, w_qkv, w_out, b_out):
    bf16 = ml_dtypes.bfloat16
    wq = np.array(w_qkv, dtype=np.float32, copy=True)
    wq[:HID] *= SCALE  # fold attention scale into the q projection
    wqkvT = np.ascontiguousarray(wq.T).astype(bf16)  # [256, 1536]
    woutT = np.ascontiguousarray(
        w_out.T.reshape(4, 128, DIM).transpose(1, 0, 2)
    ).astype(bf16)  # [128, 4, 256]
    bias = np.ascontiguousarray(
        b_out.astype(np.float32).reshape(DIM // 128, 128).T
    )  # [128, 2]
    in_maps = []
    for i in range(N_CORES):
        in_maps.append(
            {
                "x": x[i].astype(bf16),
                "wqkvT": wqkvT,
                "woutT": woutT,
                "bias": bias,
            }
        )
    return in_maps


def kernel(x, w_qkv, w_out, b_out, _trace=False):
    nc = _get_nc()
    in_maps = make_in_maps(x, w_qkv, w_out, b_out)
    res = run_bass_kernel_spmd(
        nc, in_maps, core_ids=list(range(N_CORES)), trace=_trace
    )
    out = np.stack([res.results[i]["out"] for i in range(N_CORES)], axis=0)
    if _trace:
        kernel.last_exec_time_ns = res.exec_time_ns
        kernel.last_results = res
    return out


# revision 11
# speedup vs baseline: 1.0690x; 1.0690x over previous
"""Trainium2 Bass kernel for nn_Attention_15539191677265.

Single-head-dim attention block:
    qkv = w_qkv @ x ; per-head scaled dot-product attention over w=2048;
    out = w_out @ attn_out + b_out

Sharding: pure data-parallel over batch (b=8 -> 8 NeuronCores, one batch
element per core). Weights are replicated. No collectives.

Per-core algorithm (transposed-softmax scheme, all matmuls bf16):
  1. q,k = wqkvT.T @ x           ([c,o] stationary; q pre-scaled on host)
  2. vT  = x.T @ wvT             (v produced directly transposed [j, d])
  3. per head: sim^T[j,i] strips -> exp on ScalarE (no max subtraction:
     scores are ~N(0,1), exp cannot overflow in fp32/bf16 range)
  4. AV: out^T[d,i] = vT.T @ exp_strip, with a ones-column appended to vT
     so row 64 of the psum accumulates the softmax normalizer for free
  5. normalize: reciprocal(norm row) -> partition_broadcast -> multiply
  6. proj: out = woutT.T @ attn_out (per-head K=64 chunks) + bias
"""

import sys

if "/opt/trn_rl_repo" not in sys.path:
    sys.path.insert(0, "/opt/trn_rl_repo")

import numpy as np
import ml_dtypes

import concourse.bass as bass
import concourse.mybir as mybir
import concourse.tile as tile
from concourse import bacc
from concourse.bass_utils import run_bass_kernel_spmd

BF16 = mybir.dt.bfloat16
F32 = mybir.dt.float32
EXP = mybir.ActivationFunctionType.Exp

B, DIM, W = 8, 256, 2048
HEADS, DH = 8, 64
HID = HEADS * DH  # 512
SCALE = DH ** (-0.5)
N_CORES = 8

NJT = W // 128  # 16 j-tiles per head
NCT = DIM // 128  # 2 contraction chunks over channels


def build_kernel():
    nc = bacc.Bacc(None, target_bir_lowering=False)

    x_d = nc.dram_tensor("x", [DIM, W], BF16, kind="ExternalInput")
    wqkvT_d = nc.dram_tensor("wqkvT", [DIM, 3 * HID], BF16, kind="ExternalInput")
    woutT_d = nc.dram_tensor("woutT", [128, 4, DIM], BF16, kind="ExternalInput")
    bias_d = nc.dram_tensor("bias", [128, DIM // 128], F32, kind="ExternalInput")
    out_d = nc.dram_tensor("out", [DIM, W], F32, kind="ExternalOutput")

    with tile.TileContext(nc) as tc:
        with tc.tile_pool(name="pers", bufs=1) as pers:
            x_sb = pers.tile([128, NCT, W], BF16)
            wq_sb = pers.tile([128, NCT, 3 * HID], BF16)
            wo_sb = pers.tile([128, 4, DIM], BF16)
            bias_sb = pers.tile([128, DIM // 128], F32)
            q_sb = pers.tile([128, 4, W], BF16)
            k_sb = pers.tile([128, 4, W], BF16)
            vt_sb = pers.tile([128, NJT, HEADS, DH + 1], BF16)
            attout_sb = pers.tile([128, 4, W], BF16)
            out_sb = pers.tile([128, NCT, W], F32)

            xr = x_d[:].rearrange("(ct p) w -> p ct w", p=128)
            for ct in range(NCT):
                nc.sync.dma_start(out=x_sb[:, ct, :], in_=xr[:, ct, :])
            nc.sync.dma_start(
                out=wq_sb[:], in_=wqkvT_d[:].rearrange("(ct p) o -> p ct o", p=128)
            )
            nc.sync.dma_start(out=wo_sb[:], in_=woutT_d[:])
            nc.sync.dma_start(out=bias_sb[:], in_=bias_d[:])

            # ones column of vT (col 64 of each head block)
            nc.vector.memset(vt_sb[:, :, :, DH : DH + 1], 1.0)

            # ---- phase 1: q, k projections -> [128, 4, W] bf16 each ----
            with tc.tile_pool(name="qkv_ps", bufs=2, space="PSUM") as qkv_ps:
                for ot in range(8):  # o-tiles 0..3 = q, 4..7 = k
                    ps = qkv_ps.tile([128, W], F32)
                    for ct in range(NCT):
                        for ns in range(4):
                            nc.tensor.matmul(
                                ps[:, ns * 512 : (ns + 1) * 512],
                                lhsT=wq_sb[:, ct, ot * 128 : (ot + 1) * 128],
                                rhs=x_sb[:, ct, ns * 512 : (ns + 1) * 512],
                                start=(ct == 0),
                                stop=(ct == NCT - 1),
                            )
                    dst = q_sb if ot < 4 else k_sb
                    nc.vector.tensor_copy(out=dst[:, ot % 4, :], in_=ps[:])

            # ---- phase 2: vT[j, hd] = x.T @ wvT  (+ones col kept) ----
            with tc.tile_pool(name="vt_ps", bufs=2, space="PSUM") as vt_ps:
                for jt in range(NJT):
                    ps = vt_ps.tile([128, HID], F32)
                    for ct in range(NCT):
                        nc.tensor.matmul(
                            ps[:],
                            lhsT=x_sb[:, ct, jt * 128 : (jt + 1) * 128],
                            rhs=wq_sb[:, ct, 2 * HID : 3 * HID],
                            start=(ct == 0),
                            stop=(ct == NCT - 1),
                        )
                    nc.vector.tensor_copy(
                        out=vt_sb[:, jt, :, 0:DH],
                        in_=ps[:].rearrange("p (h d) -> p h d", h=HEADS),
                    )

            # ---- phase 3: attention per head ----
            with (
                tc.tile_pool(name="strip_ps", bufs=2, space="PSUM") as strip_ps,
                tc.tile_pool(name="av_ps", bufs=2, space="PSUM") as av_ps,
                tc.tile_pool(name="exp_sb", bufs=4) as exp_pool,
                tc.tile_pool(name="norm_sb", bufs=2) as norm_pool,
            ):
                for h in range(HEADS):
                    kt, koff = h // 2, (h % 2) * 64
                    avs = [
                        av_ps.tile([DH + 1, 1024], F32, name=f"av_{h}_{ih}", tag="av")
                        for ih in range(2)
                    ]
                    for jt in range(NJT):
                        # 4 sim matmuls sharing one k-tile LDWEIGHTS, then
                        # 4 AV matmuls sharing one vT LDWEIGHTS
                        strips = []
                        for ih in range(2):
                            io = ih * 1024
                            strip = strip_ps.tile(
                                [128, 1024], F32, name=f"st_{ih}", tag="st"
                            )
                            strips.append(strip)
                            for ns in range(2):
                                nc.tensor.matmul(
                                    strip[:, ns * 512 : (ns + 1) * 512],
                                    lhsT=k_sb[
                                        koff : koff + DH, kt, jt * 128 : (jt + 1) * 128
                                    ],
                                    rhs=q_sb[
                                        koff : koff + DH,
                                        kt,
                                        io + ns * 512 : io + (ns + 1) * 512,
                                    ],
                                    start=True,
                                    stop=True,
                                )
                        ess = []
                        for ih in range(2):
                            es = exp_pool.tile(
                                [128, 1024], BF16, name=f"es_{ih}", tag="es"
                            )
                            ess.append(es)
                            nc.scalar.activation(
                                out=es[:], in_=strips[ih][:], func=EXP
                            )
                        for ih in range(2):
                            for ns in range(2):
                                nc.tensor.matmul(
                                    avs[ih][:, ns * 512 : (ns + 1) * 512],
                                    lhsT=vt_sb[:, jt, h, :],
                                    rhs=ess[ih][:, ns * 512 : (ns + 1) * 512],
                                    start=(jt == 0),
                                    stop=(jt == NJT - 1),
                                )
                    for ih in range(2):
                        io = ih * 1024
                        av = avs[ih]
                        # evacuate psum right away so the av slot frees for the
                        # next head; the norm chain then runs off-critical-path
                        avc = norm_pool.tile([DH + 1, 1024], F32, tag="avc")
                        nc.vector.tensor_copy(out=avc[:], in_=av[:])
                        rec = norm_pool.tile([DH + 1, 1024], F32, tag="rec")
                        rec0 = norm_pool.tile([1, 1024], F32, tag="rec0")
                        bc = norm_pool.tile([DH, 1024], F32, tag="bc")
                        nc.vector.reciprocal(
                            out=rec[DH : DH + 1, :], in_=avc[DH : DH + 1, :]
                        )
                        # partition_broadcast only reads partition 0 -> move row
                        nc.sync.dma_start(out=rec0[:], in_=rec[DH : DH + 1, :])
                        nc.gpsimd.partition_broadcast(
                            bc[:], rec0[0:1, :], channels=DH
                        )
                        if h % 2 == 0:
                            nc.vector.tensor_mul(
                                out=attout_sb[0:DH, h // 2, io : io + 1024],
                                in0=avc[0:DH, :],
                                in1=bc[:],
                            )
                        else:
                            # odd heads land on partitions 64..127: DVE cannot
                            # shift partitions, so write via a bounce + DMA
                            atmp = norm_pool.tile([DH, 1024], BF16, tag="atmp")
                            nc.vector.tensor_mul(
                                out=atmp[:], in0=avc[0:DH, :], in1=bc[:]
                            )
                            nc.sync.dma_start(
                                out=attout_sb[DH:128, h // 2, io : io + 1024],
                                in_=atmp[:],
                            )

            # ---- phase 4: output projection + bias (K=128 head pairs) ----
            outr = out_d[:].rearrange("(ct p) w -> p ct w", p=128)
            with tc.tile_pool(name="proj_ps", bufs=2, space="PSUM") as proj_ps:
                for ot in range(NCT):
                    for wh in range(2):
                        wo = wh * 1024
                        ps = proj_ps.tile([128, 1024], F32)
                        for kc in range(4):
                            for ns in range(2):
                                nc.tensor.matmul(
                                    ps[:, ns * 512 : (ns + 1) * 512],
                                    lhsT=wo_sb[:, kc, ot * 128 : (ot + 1) * 128],
                                    rhs=attout_sb[
                                        :, kc, wo + ns * 512 : wo + (ns + 1) * 512
                                    ],
                                    start=(kc == 0),
                                    stop=(kc == 3),
                                )
                        nc.vector.tensor_scalar_add(
                            out=out_sb[:, ot, wo : wo + 1024],
                            in0=ps[:],
                            scalar1=bias_sb[:, ot : ot + 1],
                        )
                        nc.sync.dma_start(
                            out=outr[:, ot, wo : wo + 1024],
                            in_=out_sb[:, ot, wo : wo + 1024],
                        )

    nc.compile()
    return nc


_NC_CACHE = None


def _get_nc():
    global _NC_CACHE
    if _NC_CACHE is None:
        _NC_CACHE = build_kernel()
    return _NC_CACHE


def make_in_maps(x, w_qkv, w_out, b_out):
    bf16 = ml_dtypes.bfloat16
    wq = np.array(w_qkv, dtype=np.float32, copy=True)
    wq[:HID] *= SCALE  # fold attention scale into the q projection
    wqkvT = np.ascontiguousarray(wq.T).astype(bf16)  # [256, 1536]
    woutT = np.ascontiguousarray(
        w_out.T.reshape(4, 128, DIM).transpose(1, 0, 2)
    ).astype(bf16)  # [128, 4, 256]
    bias = np.ascontiguousarray(
        b_out.astype(np.float32).reshape(DIM // 128, 128).T
    )  # [128, 2]
    in_maps = []
    for i in range(N_CORES):
        in_maps.append(
            {
                "x": x[i].astype(bf16),
                "wqkvT": wqkvT,
                "woutT": woutT,
                "bias": bias,
            }
        )
    return in_maps


def kernel(x, w_qkv, w_out, b_out, _trace=False):
    nc = _get_nc()
    in_maps = make_in_maps(x, w_qkv, w_out, b_out)
    res = run_bass_kernel_spmd(
        nc, in_maps, core_ids=list(range(N_CORES)), trace=_trace
    )
    out = np.stack([res.results[i]["out"] for i in range(N_CORES)], axis=0)
    if _trace:
        kernel.last_exec_time_ns = res.exec_time_ns
        kernel.last_results = res
    return out


# revision 12
# speedup vs baseline: 1.0707x; 1.0016x over previous
"""Trainium2 Bass kernel for nn_Attention_15539191677265.

Single-head-dim attention block:
    qkv = w_qkv @ x ; per-head scaled dot-product attention over w=2048;
    out = w_out @ attn_out + b_out

Sharding: pure data-parallel over batch (b=8 -> 8 NeuronCores, one batch
element per core). Weights are replicated. No collectives.

Per-core algorithm (transposed-softmax scheme, all matmuls bf16):
  1. q,k = wqkvT.T @ x           ([c,o] stationary; q pre-scaled on host)
  2. vT  = x.T @ wvT             (v produced directly transposed [j, d])
  3. per head: sim^T[j,i] strips -> exp on ScalarE (no max subtraction:
     scores are ~N(0,1), exp cannot overflow in fp32/bf16 range)
  4. AV: out^T[d,i] = vT.T @ exp_strip, with a ones-column appended to vT
     so row 64 of the psum accumulates the softmax normalizer for free
  5. normalize: reciprocal(norm row) -> partition_broadcast -> multiply
  6. proj: out = woutT.T @ attn_out (per-head K=64 chunks) + bias
"""

import sys

if "/opt/trn_rl_repo" not in sys.path:
    sys.path.insert(0, "/opt/trn_rl_repo")

import numpy as np
import ml_dtypes

import concourse.bass as bass
import concourse.mybir as mybir
import concourse.tile as tile
from concourse import bacc
from concourse.bass_utils import run_bass_kernel_spmd

BF16 = mybir.dt.bfloat16
F32 = mybir.dt.float32
EXP = mybir.ActivationFunctionType.Exp

B, DIM, W = 8, 256, 2048
HEADS, DH = 8, 64
HID = HEADS * DH  # 512
SCALE = DH ** (-0.5)
N_CORES = 8

NJT = W // 128  # 16 j-tiles per head
NCT = DIM // 128  # 2 contraction chunks over channels


def build_kernel():
    nc = bacc.Bacc(None, target_bir_lowering=False)

    x_d = nc.dram_tensor("x", [DIM, W], BF16, kind="ExternalInput")
    wqkvT_d = nc.dram_tensor("wqkvT", [DIM, 3 * HID], BF16, kind="ExternalInput")
    woutT_d = nc.dram_tensor("woutT", [128, 4, DIM], BF16, kind="ExternalInput")
    bias_d = nc.dram_tensor("bias", [128, DIM // 128], F32, kind="ExternalInput")
    out_d = nc.dram_tensor("out", [DIM, W], F32, kind="ExternalOutput")

    with tile.TileContext(nc) as tc:
        with tc.tile_pool(name="pers", bufs=1) as pers:
            x_sb = pers.tile([128, NCT, W], BF16)
            wq_sb = pers.tile([128, NCT, 3 * HID], BF16)
            wo_sb = pers.tile([128, 4, DIM], BF16)
            bias_sb = pers.tile([128, DIM // 128], F32)
            q_sb = pers.tile([128, 4, W], BF16)
            k_sb = pers.tile([128, 4, W], BF16)
            vt_sb = pers.tile([128, NJT, HEADS, DH + 1], BF16)
            attout_sb = [
                pers.tile([128, W], BF16, name=f"attout_{kc}", tag=f"attout{kc}")
                for kc in range(4)
            ]
            out_sb = pers.tile([128, NCT, W], F32)

            xr = x_d[:].rearrange("(ct p) w -> p ct w", p=128)
            for ct in range(NCT):
                for wh in range(2):
                    nc.sync.dma_start(
                        out=x_sb[:, ct, wh * 1024 : (wh + 1) * 1024],
                        in_=xr[:, ct, wh * 1024 : (wh + 1) * 1024],
                    )
            nc.sync.dma_start(
                out=wq_sb[:], in_=wqkvT_d[:].rearrange("(ct p) o -> p ct o", p=128)
            )
            nc.sync.dma_start(out=wo_sb[:], in_=woutT_d[:])
            nc.sync.dma_start(out=bias_sb[:], in_=bias_d[:])

            # ones column of vT (col 64 of each head block)
            nc.vector.memset(vt_sb[:, :, :, DH : DH + 1], 1.0)

            # ---- phase 1: q, k projections -> [128, 4, W] bf16 each ----
            with tc.tile_pool(name="qkv_ps", bufs=2, space="PSUM") as qkv_ps:
                for ot in range(8):  # o-tiles 0..3 = q, 4..7 = k
                    ps = qkv_ps.tile([128, W], F32)
                    for ct in range(NCT):
                        for ns in range(4):
                            nc.tensor.matmul(
                                ps[:, ns * 512 : (ns + 1) * 512],
                                lhsT=wq_sb[:, ct, ot * 128 : (ot + 1) * 128],
                                rhs=x_sb[:, ct, ns * 512 : (ns + 1) * 512],
                                start=(ct == 0),
                                stop=(ct == NCT - 1),
                            )
                    dst = q_sb if ot < 4 else k_sb
                    nc.vector.tensor_copy(out=dst[:, ot % 4, :], in_=ps[:])

            # ---- phase 2: vT[j, hd] = x.T @ wvT  (+ones col kept) ----
            with tc.tile_pool(name="vt_ps", bufs=2, space="PSUM") as vt_ps:
                for jt in range(NJT):
                    ps = vt_ps.tile([128, HID], F32)
                    for ct in range(NCT):
                        nc.tensor.matmul(
                            ps[:],
                            lhsT=x_sb[:, ct, jt * 128 : (jt + 1) * 128],
                            rhs=wq_sb[:, ct, 2 * HID : 3 * HID],
                            start=(ct == 0),
                            stop=(ct == NCT - 1),
                        )
                    nc.vector.tensor_copy(
                        out=vt_sb[:, jt, :, 0:DH],
                        in_=ps[:].rearrange("p (h d) -> p h d", h=HEADS),
                    )

            # ---- phase 3: attention per head ----
            with (
                tc.tile_pool(name="strip_ps", bufs=2, space="PSUM") as strip_ps,
                tc.tile_pool(name="av_ps", bufs=2, space="PSUM") as av_ps,
                tc.tile_pool(name="exp_sb", bufs=4) as exp_pool,
                tc.tile_pool(name="norm_sb", bufs=2) as norm_pool,
            ):
                for h in range(HEADS):
                    kt, koff = h // 2, (h % 2) * 64
                    avs = [
                        av_ps.tile([DH + 1, 1024], F32, name=f"av_{h}_{ih}", tag="av")
                        for ih in range(2)
                    ]
                    for jt in range(NJT):
                        # 4 sim matmuls sharing one k-tile LDWEIGHTS, then
                        # 4 AV matmuls sharing one vT LDWEIGHTS
                        strips = []
                        for ih in range(2):
                            io = ih * 1024
                            strip = strip_ps.tile(
                                [128, 1024], F32, name=f"st_{ih}", tag="st"
                            )
                            strips.append(strip)
                            for ns in range(2):
                                nc.tensor.matmul(
                                    strip[:, ns * 512 : (ns + 1) * 512],
                                    lhsT=k_sb[
                                        koff : koff + DH, kt, jt * 128 : (jt + 1) * 128
                                    ],
                                    rhs=q_sb[
                                        koff : koff + DH,
                                        kt,
                                        io + ns * 512 : io + (ns + 1) * 512,
                                    ],
                                    start=True,
                                    stop=True,
                                )
                        ess = []
                        for ih in range(2):
                            es = exp_pool.tile(
                                [128, 1024], BF16, name=f"es_{ih}", tag="es"
                            )
                            ess.append(es)
                            nc.scalar.activation(
                                out=es[:], in_=strips[ih][:], func=EXP
                            )
                        for ih in range(2):
                            for ns in range(2):
                                nc.tensor.matmul(
                                    avs[ih][:, ns * 512 : (ns + 1) * 512],
                                    lhsT=vt_sb[:, jt, h, :],
                                    rhs=ess[ih][:, ns * 512 : (ns + 1) * 512],
                                    start=(jt == 0),
                                    stop=(jt == NJT - 1),
                                )
                    for ih in range(2):
                        io = ih * 1024
                        av = avs[ih]
                        # evacuate psum right away so the av slot frees for the
                        # next head; the norm chain then runs off-critical-path
                        avc = norm_pool.tile([DH + 1, 1024], F32, tag="avc")
                        nc.vector.tensor_copy(out=avc[:], in_=av[:])
                        rec = norm_pool.tile([DH + 1, 1024], F32, tag="rec")
                        rec0 = norm_pool.tile([1, 1024], F32, tag="rec0")
                        bc = norm_pool.tile([DH, 1024], F32, tag="bc")
                        nc.vector.reciprocal(
                            out=rec[DH : DH + 1, :], in_=avc[DH : DH + 1, :]
                        )
                        # partition_broadcast only reads partition 0 -> move row
                        nc.sync.dma_start(out=rec0[:], in_=rec[DH : DH + 1, :])
                        nc.gpsimd.partition_broadcast(
                            bc[:], rec0[0:1, :], channels=DH
                        )
                        if h % 2 == 0:
                            nc.vector.tensor_mul(
                                out=attout_sb[h // 2][0:DH, io : io + 1024],
                                in0=avc[0:DH, :],
                                in1=bc[:],
                            )
                        else:
                            # odd heads land on partitions 64..127: DVE cannot
                            # shift partitions, so write via a bounce + DMA
                            atmp = norm_pool.tile([DH, 1024], BF16, tag="atmp")
                            nc.vector.tensor_mul(
                                out=atmp[:], in0=avc[0:DH, :], in1=bc[:]
                            )
                            nc.sync.dma_start(
                                out=attout_sb[h // 2][DH:128, io : io + 1024],
                                in_=atmp[:],
                            )

            # ---- phase 4: output projection + bias (K=128 head pairs) ----
            outr = out_d[:].rearrange("(ct p) w -> p ct w", p=128)
            with tc.tile_pool(name="proj_ps", bufs=2, space="PSUM") as proj_ps:
                for ot in range(NCT):
                    for wh in range(2):
                        wo = wh * 1024
                        ps = proj_ps.tile([128, 1024], F32)
                        for kc in range(4):
                            for ns in range(2):
                                nc.tensor.matmul(
                                    ps[:, ns * 512 : (ns + 1) * 512],
                                    lhsT=wo_sb[:, kc, ot * 128 : (ot + 1) * 128],
                                    rhs=attout_sb[kc][
                                        :, wo + ns * 512 : wo + (ns + 1) * 512
                                    ],
                                    start=(kc == 0),
                                    stop=(kc == 3),
                                )
                        nc.vector.tensor_scalar_add(
                            out=out_sb[:, ot, wo : wo + 1024],
                            in0=ps[:],
                            scalar1=bias_sb[:, ot : ot + 1],
                        )
                        nc.sync.dma_start(
                            out=outr[:, ot, wo : wo + 1024],
                            in_=out_sb[:, ot, wo : wo + 1024],
                        )

    nc.compile()
    return nc


_NC_CACHE = None


def _get_nc():
    global _NC_CACHE
    if _NC_CACHE is None:
        _NC_CACHE = build_kernel()
    return _NC_CACHE


def make_in_maps(x, w_qkv, w_out, b_out):
    bf16 = ml_dtypes.bfloat16
    wq = np.array(w_qkv, dtype=np.float32, copy=True)
    wq[:HID] *= SCALE  # fold attention scale into the q projection
    wqkvT = np.ascontiguousarray(wq.T).astype(bf16)  # [256, 1536]
    woutT = np.ascontiguousarray(
        w_out.T.reshape(4, 128, DIM).transpose(1, 0, 2)
    ).astype(bf16)  # [128, 4, 256]
    bias = np.ascontiguousarray(
        b_out.astype(np.float32).reshape(DIM // 128, 128).T
    )  # [128, 2]
    in_maps = []
    for i in range(N_CORES):
        in_maps.append(
            {
                "x": x[i].astype(bf16),
                "wqkvT": wqkvT,
                "woutT": woutT,
                "bias": bias,
            }
        )
    return in_maps


def kernel(x, w_qkv, w_out, b_out, _trace=False):
    nc = _get_nc()
    in_maps = make_in_maps(x, w_qkv, w_out, b_out)
    res = run_bass_kernel_spmd(
        nc, in_maps, core_ids=list(range(N_CORES)), trace=_trace
    )
    out = np.stack([res.results[i]["out"] for i in range(N_CORES)], axis=0)
    if _trace:
        kernel.last_exec_time_ns = res.exec_time_ns
        kernel.last_results = res
    return out


# revision 14
# speedup vs baseline: 1.0867x; 1.0150x over previous
"""Trainium2 Bass kernel for nn_Attention_15539191677265.

Single-head-dim attention block:
    qkv = w_qkv @ x ; per-head scaled dot-product attention over w=2048;
    out = w_out @ attn_out + b_out

Sharding: pure data-parallel over batch (b=8 -> 8 NeuronCores, one batch
element per core). Weights are replicated. No collectives.

Per-core algorithm (transposed-softmax scheme, all matmuls bf16):
  1. q,k = wqkvT.T @ x           ([c,o] stationary; q pre-scaled on host)
  2. vT  = x.T @ wvT             (v produced directly transposed [j, d])
  3. per head: sim^T[j,i] strips -> exp on ScalarE (no max subtraction:
     scores are ~N(0,1), exp cannot overflow in fp32/bf16 range)
  4. AV: out^T[d,i] = vT.T @ exp_strip, with a ones-column appended to vT
     so row 64 of the psum accumulates the softmax normalizer for free
  5. normalize: reciprocal(norm row) -> partition_broadcast -> multiply
  6. proj: out = woutT.T @ attn_out (per-head K=64 chunks) + bias
"""

import sys

if "/opt/trn_rl_repo" not in sys.path:
    sys.path.insert(0, "/opt/trn_rl_repo")

import numpy as np
import ml_dtypes

import concourse.bass as bass
import concourse.mybir as mybir
import concourse.tile as tile
from concourse import bacc
from concourse.bass_utils import run_bass_kernel_spmd

BF16 = mybir.dt.bfloat16
F32 = mybir.dt.float32
EXP = mybir.ActivationFunctionType.Exp

B, DIM, W = 8, 256, 2048
HEADS, DH = 8, 64
HID = HEADS * DH  # 512
SCALE = DH ** (-0.5)
N_CORES = 8

NJT = W // 128  # 16 j-tiles per head
NCT = DIM // 128  # 2 contraction chunks over channels


def build_kernel():
    nc = bacc.Bacc(None, target_bir_lowering=False)

    x_d = nc.dram_tensor("x", [DIM, W], BF16, kind="ExternalInput")
    wqkvT_d = nc.dram_tensor("wqkvT", [DIM, 3 * HID], BF16, kind="ExternalInput")
    woutT_d = nc.dram_tensor("woutT", [128, 4, DIM], BF16, kind="ExternalInput")
    bias_d = nc.dram_tensor("bias", [128, DIM // 128], F32, kind="ExternalInput")
    out_d = nc.dram_tensor("out", [DIM, W], F32, kind="ExternalOutput")

    with tile.TileContext(nc) as tc:
        with tc.tile_pool(name="pers", bufs=1) as pers:
            x_sb = pers.tile([128, NCT, W], BF16)
            wq_sb = pers.tile([128, NCT, 3 * HID], BF16)
            wo_sb = pers.tile([128, 4, DIM], BF16)
            bias_sb = pers.tile([128, DIM // 128], F32)
            q_sb = pers.tile([128, 4, W], BF16)
            k_sb = pers.tile([128, 4, W], BF16)
            vt_sb = pers.tile([128, NJT, HEADS, DH + 1], BF16)
            attout_sb = [
                pers.tile([128, W], BF16, name=f"attout_{kc}", tag=f"attout{kc}")
                for kc in range(4)
            ]
            out_sb = pers.tile([128, NCT, W], F32)

            xr = x_d[:].rearrange("(ct p) w -> p ct w", p=128)
            for ct in range(NCT):
                for wh in range(2):
                    nc.sync.dma_start(
                        out=x_sb[:, ct, wh * 1024 : (wh + 1) * 1024],
                        in_=xr[:, ct, wh * 1024 : (wh + 1) * 1024],
                    )
            nc.sync.dma_start(
                out=wq_sb[:], in_=wqkvT_d[:].rearrange("(ct p) o -> p ct o", p=128)
            )
            nc.sync.dma_start(out=wo_sb[:], in_=woutT_d[:])
            nc.sync.dma_start(out=bias_sb[:], in_=bias_d[:])

            # ones column of vT (col 64 of each head block)
            nc.vector.memset(vt_sb[:, :, :, DH : DH + 1], 1.0)

            # ---- phase 1: q, k projections -> [128, 4, W] bf16 each ----
            with tc.tile_pool(name="qkv_ps", bufs=2, space="PSUM") as qkv_ps:
                for ot in range(8):  # o-tiles 0..3 = q, 4..7 = k
                    ps = qkv_ps.tile([128, W], F32)
                    for ct in range(NCT):
                        for ns in range(4):
                            nc.tensor.matmul(
                                ps[:, ns * 512 : (ns + 1) * 512],
                                lhsT=wq_sb[:, ct, ot * 128 : (ot + 1) * 128],
                                rhs=x_sb[:, ct, ns * 512 : (ns + 1) * 512],
                                start=(ct == 0),
                                stop=(ct == NCT - 1),
                            )
                    dst = q_sb if ot < 4 else k_sb
                    nc.vector.tensor_copy(out=dst[:, ot % 4, :], in_=ps[:])

            # ---- phase 2: vT[j, hd] = x.T @ wvT  (+ones col kept) ----
            with tc.tile_pool(name="vt_ps", bufs=2, space="PSUM") as vt_ps:
                for jt in range(NJT):
                    ps = vt_ps.tile([128, HID], F32)
                    for ct in range(NCT):
                        nc.tensor.matmul(
                            ps[:],
                            lhsT=x_sb[:, ct, jt * 128 : (jt + 1) * 128],
                            rhs=wq_sb[:, ct, 2 * HID : 3 * HID],
                            start=(ct == 0),
                            stop=(ct == NCT - 1),
                        )
                    nc.vector.tensor_copy(
                        out=vt_sb[:, jt, :, 0:DH],
                        in_=ps[:].rearrange("p (h d) -> p h d", h=HEADS),
                    )

            # ---- phase 3: attention per head ----
            with (
                tc.tile_pool(name="strip_ps", bufs=2, space="PSUM") as strip_ps,
                tc.tile_pool(name="av_ps", bufs=2, space="PSUM") as av_ps,
                tc.tile_pool(name="exp_sb", bufs=4) as exp_pool,
                tc.tile_pool(name="norm_sb", bufs=2) as norm_pool,
            ):
                for h in range(HEADS):
                    kt, koff = h // 2, (h % 2) * 64
                    avs = [
                        av_ps.tile([DH + 1, 1024], F32, name=f"av_{h}_{ih}", tag="av")
                        for ih in range(2)
                    ]
                    for jt in range(NJT):
                        # 4 sim matmuls sharing one k-tile LDWEIGHTS, then
                        # 4 AV matmuls sharing one vT LDWEIGHTS
                        strips = []
                        for ih in range(2):
                            io = ih * 1024
                            strip = strip_ps.tile(
                                [128, 1024], F32, name=f"st_{ih}", tag="st"
                            )
                            strips.append(strip)
                            for ns in range(2):
                                nc.tensor.matmul(
                                    strip[:, ns * 512 : (ns + 1) * 512],
                                    lhsT=k_sb[
                                        koff : koff + DH, kt, jt * 128 : (jt + 1) * 128
                                    ],
                                    rhs=q_sb[
                                        koff : koff + DH,
                                        kt,
                                        io + ns * 512 : io + (ns + 1) * 512,
                                    ],
                                    start=True,
                                    stop=True,
                                )
                        ess = []
                        for ih in range(2):
                            es = exp_pool.tile(
                                [128, 1024], BF16, name=f"es_{ih}", tag="es"
                            )
                            ess.append(es)
                            nc.scalar.activation(
                                out=es[:], in_=strips[ih][:], func=EXP
                            )
                        for ih in range(2):
                            for ns in range(2):
                                nc.tensor.matmul(
                                    avs[ih][:, ns * 512 : (ns + 1) * 512],
                                    lhsT=vt_sb[:, jt, h, :],
                                    rhs=ess[ih][:, ns * 512 : (ns + 1) * 512],
                                    start=(jt == 0),
                                    stop=(jt == NJT - 1),
                                )
                    for ih in range(2):
                        io = ih * 1024
                        av = avs[ih]
                        # evacuate psum right away so the av slot frees for the
                        # next head; the norm chain then runs off-critical-path
                        avc = norm_pool.tile([DH + 1, 1024], F32, tag="avc")
                        nc.vector.tensor_copy(out=avc[:], in_=av[:])
                        rec0 = norm_pool.tile([1, 1024], F32, tag="rec0")
                        bcn = norm_pool.tile([DH, 1024], F32, tag="bcn")
                        bc = norm_pool.tile([DH, 1024], F32, tag="bc")
                        # partition_broadcast + custom-DVE ops only work from
                        # partition 0 -> DMA the raw norm row there first
                        nc.sync.dma_start(out=rec0[:], in_=avc[DH : DH + 1, :])
                        nc.gpsimd.partition_broadcast(
                            bcn[:], rec0[0:1, :], channels=DH
                        )
                        nc.vector.reciprocal_approx_fast(out=bc[:], in_=bcn[:])
                        if h % 2 == 0:
                            nc.vector.tensor_mul(
                                out=attout_sb[h // 2][0:DH, io : io + 1024],
                                in0=avc[0:DH, :],
                                in1=bc[:],
                            )
                        else:
                            # odd heads land on partitions 64..127: DVE cannot
                            # shift partitions, so write via a bounce + DMA
                            atmp = norm_pool.tile([DH, 1024], BF16, tag="atmp")
                            nc.vector.tensor_mul(
                                out=atmp[:], in0=avc[0:DH, :], in1=bc[:]
                            )
                            nc.sync.dma_start(
                                out=attout_sb[h // 2][DH:128, io : io + 1024],
                                in_=atmp[:],
                            )

            # ---- phase 4: output projection + bias (K=128 head pairs) ----
            outr = out_d[:].rearrange("(ct p) w -> p ct w", p=128)
            with tc.tile_pool(name="proj_ps", bufs=2, space="PSUM") as proj_ps:
                for ot in range(NCT):
                    for wh in range(2):
                        wo = wh * 1024
                        ps = proj_ps.tile([128, 1024], F32)
                        for kc in range(4):
                            for ns in range(2):
                                nc.tensor.matmul(
                                    ps[:, ns * 512 : (ns + 1) * 512],
                                    lhsT=wo_sb[:, kc, ot * 128 : (ot + 1) * 128],
                                    rhs=attout_sb[kc][
                                        :, wo + ns * 512 : wo + (ns + 1) * 512
                                    ],
                                    start=(kc == 0),
                                    stop=(kc == 3),
                                )
                        nc.vector.tensor_scalar_add(
                            out=out_sb[:, ot, wo : wo + 1024],
                            in0=ps[:],
                            scalar1=bias_sb[:, ot : ot + 1],
                        )
                        nc.sync.dma_start(
                            out=outr[:, ot, wo : wo + 1024],
                            in_=out_sb[:, ot, wo : wo + 1024],
                        )

    nc.compile()
    return nc


_NC_CACHE = None


def _get_nc():
    global _NC_CACHE
    if _NC_CACHE is None:
        _NC_CACHE = build_kernel()
    return _NC_CACHE


def make_in_maps(x, w_qkv, w_out, b_out):
    bf16 = ml_dtypes.bfloat16
    wq = np.array(w_qkv, dtype=np.float32, copy=True)
    wq[:HID] *= SCALE  # fold attention scale into the q projection
    wqkvT = np.ascontiguousarray(wq.T).astype(bf16)  # [256, 1536]
    woutT = np.ascontiguousarray(
        w_out.T.reshape(4, 128, DIM).transpose(1, 0, 2)
    ).astype(bf16)  # [128, 4, 256]
    bias = np.ascontiguousarray(
        b_out.astype(np.float32).reshape(DIM // 128, 128).T
    )  # [128, 2]
    in_maps = []
    for i in range(N_CORES):
        in_maps.append(
            {
                "x": x[i].astype(bf16),
                "wqkvT": wqkvT,
                "woutT": woutT,
                "bias": bias,
            }
        )
    return in_maps


def kernel(x, w_qkv, w_out, b_out, _trace=False):
    nc = _get_nc()
    in_maps = make_in_maps(x, w_qkv, w_out, b_out)
    res = run_bass_kernel_spmd(
        nc, in_maps, core_ids=list(range(N_CORES)), trace=_trace
    )
    out = np.stack([res.results[i]["out"] for i in range(N_CORES)], axis=0)
    if _trace:
        kernel.last_exec_time_ns = res.exec_time_ns
        kernel.last_results = res
    return out


# revision 15
# speedup vs baseline: 1.6481x; 1.5167x over previous
"""Trainium2 Bass kernel for nn_Attention_15539191677265.

Single-head-dim attention block:
    qkv = w_qkv @ x ; per-head scaled dot-product attention over w=2048;
    out = w_out @ attn_out + b_out

Sharding: pure data-parallel over batch (b=8 -> 8 NeuronCores, one batch
element per core). Weights are replicated. No collectives.

Per-core algorithm (transposed-softmax scheme, all matmuls bf16):
  1. q,k = wqkvT.T @ x           ([c,o] stationary; q pre-scaled on host)
  2. vT  = x.T @ wvT             (v produced directly transposed [j, d])
  3. per head: sim^T[j,i] strips -> exp on ScalarE (no max subtraction:
     scores are ~N(0,1), exp cannot overflow in fp32/bf16 range)
  4. AV: out^T[d,i] = vT.T @ exp_strip, with a ones-column appended to vT
     so row 64 of the psum accumulates the softmax normalizer for free
  5. normalize: reciprocal(norm row) -> partition_broadcast -> multiply
  6. proj: out = woutT.T @ attn_out (per-head K=64 chunks) + bias
"""

import sys

if "/opt/trn_rl_repo" not in sys.path:
    sys.path.insert(0, "/opt/trn_rl_repo")

import numpy as np
import ml_dtypes

import concourse.bass as bass
import concourse.mybir as mybir
import concourse.tile as tile
from concourse import bacc
from concourse.bass_utils import run_bass_kernel_spmd

BF16 = mybir.dt.bfloat16
F32 = mybir.dt.float32
EXP = mybir.ActivationFunctionType.Exp

B, DIM, W = 8, 256, 2048
HEADS, DH = 8, 64
HID = HEADS * DH  # 512
SCALE = DH ** (-0.5)
N_CORES = 8

NJT = W // 128  # 16 j-tiles per head
NCT = DIM // 128  # 2 contraction chunks over channels


def build_kernel():
    nc = bacc.Bacc(None, target_bir_lowering=False)

    x_d = nc.dram_tensor("x", [DIM, W], BF16, kind="ExternalInput")
    wqkvT_d = nc.dram_tensor("wqkvT", [DIM, 3 * HID], BF16, kind="ExternalInput")
    woutT_d = nc.dram_tensor("woutT", [128, 4, DIM], BF16, kind="ExternalInput")
    bias_d = nc.dram_tensor("bias", [128, DIM // 128], F32, kind="ExternalInput")
    out_d = nc.dram_tensor("out", [DIM, W], F32, kind="ExternalOutput")

    with tile.TileContext(nc) as tc:
        with tc.tile_pool(name="pers", bufs=1) as pers:
            x_sb = pers.tile([128, NCT, W], BF16)
            wq_sb = pers.tile([128, NCT, 3 * HID], BF16)
            wo_sb = pers.tile([128, 4, DIM], BF16)
            bias_sb = pers.tile([128, DIM // 128], F32)
            q_sb = pers.tile([128, 4, W], BF16)
            k_sb = pers.tile([128, 4, W], BF16)
            vt_sb = pers.tile([128, NJT, HEADS, DH + 1], BF16)
            attout_sb = [
                pers.tile([128, W], BF16, name=f"attout_{kc}", tag=f"attout{kc}")
                for kc in range(4)
            ]
            out_sb = pers.tile([128, NCT, W], F32)

            xr = x_d[:].rearrange("(ct p) w -> p ct w", p=128)
            for ct in range(NCT):
                for wh in range(2):
                    nc.sync.dma_start(
                        out=x_sb[:, ct, wh * 1024 : (wh + 1) * 1024],
                        in_=xr[:, ct, wh * 1024 : (wh + 1) * 1024],
                    )
            nc.sync.dma_start(
                out=wq_sb[:], in_=wqkvT_d[:].rearrange("(ct p) o -> p ct o", p=128)
            )
            nc.sync.dma_start(out=wo_sb[:], in_=woutT_d[:])
            nc.sync.dma_start(out=bias_sb[:], in_=bias_d[:])

            # ones column of vT (col 64 of each head block)
            nc.vector.memset(vt_sb[:, :, :, DH : DH + 1], 1.0)
            # warm the ACT exp table set while qkv matmuls run
            warm = pers.tile([1, 1], F32)
            nc.vector.memset(warm[:], 0.0)
            nc.scalar.activation(out=warm[:], in_=warm[:], func=EXP)

            # ---- phase 1: q, k projections -> [128, 4, W] bf16 each ----
            with tc.tile_pool(name="qkv_ps", bufs=2, space="PSUM") as qkv_ps:
                for ot in range(8):  # o-tiles 0..3 = q, 4..7 = k
                    ps = qkv_ps.tile([128, W], F32)
                    for ct in range(NCT):
                        for ns in range(4):
                            nc.tensor.matmul(
                                ps[:, ns * 512 : (ns + 1) * 512],
                                lhsT=wq_sb[:, ct, ot * 128 : (ot + 1) * 128],
                                rhs=x_sb[:, ct, ns * 512 : (ns + 1) * 512],
                                start=(ct == 0),
                                stop=(ct == NCT - 1),
                            )
                    dst = q_sb if ot < 4 else k_sb
                    nc.vector.tensor_copy(out=dst[:, ot % 4, :], in_=ps[:])

            # ---- phase 2: vT[j, hd] = x.T @ wvT  (+ones col kept) ----
            with tc.tile_pool(name="vt_ps", bufs=2, space="PSUM") as vt_ps:
                for jt in range(NJT):
                    ps = vt_ps.tile([128, HID], F32)
                    for ct in range(NCT):
                        nc.tensor.matmul(
                            ps[:],
                            lhsT=x_sb[:, ct, jt * 128 : (jt + 1) * 128],
                            rhs=wq_sb[:, ct, 2 * HID : 3 * HID],
                            start=(ct == 0),
                            stop=(ct == NCT - 1),
                        )
                    nc.vector.tensor_copy(
                        out=vt_sb[:, jt, :, 0:DH],
                        in_=ps[:].rearrange("p (h d) -> p h d", h=HEADS),
                    )

            # ---- phase 3: attention per head ----
            with (
                tc.tile_pool(name="strip_ps", bufs=2, space="PSUM") as strip_ps,
                tc.tile_pool(name="av_ps", bufs=2, space="PSUM") as av_ps,
                tc.tile_pool(name="exp_sb", bufs=6) as exp_pool,
                tc.tile_pool(name="norm_sb", bufs=3) as norm_pool,
            ):
                for h in range(HEADS):
                    kt, koff = h // 2, (h % 2) * 64
                    avs = [
                        av_ps.tile([DH + 1, 1024], F32, name=f"av_{h}_{ih}", tag="av")
                        for ih in range(2)
                    ]
                    for jt in range(NJT):
                        # 4 sim matmuls sharing one k-tile LDWEIGHTS, then
                        # 4 AV matmuls sharing one vT LDWEIGHTS
                        strips = []
                        for ih in range(2):
                            io = ih * 1024
                            strip = strip_ps.tile(
                                [128, 1024], F32, name=f"st_{ih}", tag="st"
                            )
                            strips.append(strip)
                            for ns in range(2):
                                nc.tensor.matmul(
                                    strip[:, ns * 512 : (ns + 1) * 512],
                                    lhsT=k_sb[
                                        koff : koff + DH, kt, jt * 128 : (jt + 1) * 128
                                    ],
                                    rhs=q_sb[
                                        koff : koff + DH,
                                        kt,
                                        io + ns * 512 : io + (ns + 1) * 512,
                                    ],
                                    start=True,
                                    stop=True,
                                )
                        ess = []
                        for ih in range(2):
                            es = exp_pool.tile(
                                [128, 1024], BF16, name=f"es_{ih}", tag="es"
                            )
                            ess.append(es)
                            nc.scalar.activation(
                                out=es[:], in_=strips[ih][:], func=EXP
                            )
                        for ih in range(2):
                            for ns in range(2):
                                nc.tensor.matmul(
                                    avs[ih][:, ns * 512 : (ns + 1) * 512],
                                    lhsT=vt_sb[:, jt, h, :],
                                    rhs=ess[ih][:, ns * 512 : (ns + 1) * 512],
                                    start=(jt == 0),
                                    stop=(jt == NJT - 1),
                                )
                    for ih in range(2):
                        io = ih * 1024
                        av = avs[ih]
                        # evacuate psum right away so the av slot frees for the
                        # next head; the norm chain then runs off-critical-path
                        avc = norm_pool.tile([DH + 1, 1024], F32, tag="avc")
                        nc.vector.tensor_copy(out=avc[:], in_=av[:])
                        rec0 = norm_pool.tile([1, 1024], F32, tag="rec0")
                        bcn = norm_pool.tile([DH, 1024], F32, tag="bcn")
                        bc = norm_pool.tile([DH, 1024], F32, tag="bc")
                        # partition_broadcast + custom-DVE ops only work from
                        # partition 0 -> DMA the raw norm row there first
                        nc.sync.dma_start(out=rec0[:], in_=avc[DH : DH + 1, :])
                        nc.gpsimd.partition_broadcast(
                            bcn[:], rec0[0:1, :], channels=DH
                        )
                        nc.vector.reciprocal_approx_fast(out=bc[:], in_=bcn[:])
                        if h % 2 == 0:
                            nc.vector.tensor_mul(
                                out=attout_sb[h // 2][0:DH, io : io + 1024],
                                in0=avc[0:DH, :],
                                in1=bc[:],
                            )
                        else:
                            # odd heads land on partitions 64..127: DVE cannot
                            # shift partitions, so write via a bounce + DMA
                            atmp = norm_pool.tile([DH, 1024], BF16, tag="atmp")
                            nc.vector.tensor_mul(
                                out=atmp[:], in0=avc[0:DH, :], in1=bc[:]
                            )
                            nc.sync.dma_start(
                                out=attout_sb[h // 2][DH:128, io : io + 1024],
                                in_=atmp[:],
                            )

            # ---- phase 4: output projection + bias (K=128 head pairs) ----
            outr = out_d[:].rearrange("(ct p) w -> p ct w", p=128)
            with tc.tile_pool(name="proj_ps", bufs=2, space="PSUM") as proj_ps:
                for ot in range(NCT):
                    for wh in range(2):
                        wo = wh * 1024
                        ps = proj_ps.tile([128, 1024], F32)
                        for kc in range(4):
                            for ns in range(2):
                                nc.tensor.matmul(
                                    ps[:, ns * 512 : (ns + 1) * 512],
                                    lhsT=wo_sb[:, kc, ot * 128 : (ot + 1) * 128],
                                    rhs=attout_sb[kc][
                                        :, wo + ns * 512 : wo + (ns + 1) * 512
                                    ],
                                    start=(kc == 0),
                                    stop=(kc == 3),
                                )
                        nc.vector.tensor_scalar_add(
                            out=out_sb[:, ot, wo : wo + 1024],
                            in0=ps[:],
                            scalar1=bias_sb[:, ot : ot + 1],
                        )
                        nc.sync.dma_start(
                            out=outr[:, ot, wo : wo + 1024],
                            in_=out_sb[:, ot, wo : wo + 1024],
                        )

    nc.compile()
    return nc


_NC_CACHE = None


def _get_nc():
    global _NC_CACHE
    if _NC_CACHE is None:
        _NC_CACHE = build_kernel()
    return _NC_CACHE


def make_in_maps(x, w_qkv, w_out, b_out):
    bf16 = ml_dtypes.bfloat16
    wq = np.array(w_qkv, dtype=np.float32, copy=True)
    wq[:HID] *= SCALE  # fold attention scale into the q projection
    wqkvT = np.ascontiguousarray(wq.T).astype(bf16)  # [256, 1536]
    woutT = np.ascontiguousarray(
        w_out.T.reshape(4, 128, DIM).transpose(1, 0, 2)
    ).astype(bf16)  # [128, 4, 256]
    bias = np.ascontiguousarray(
        b_out.astype(np.float32).reshape(DIM // 128, 128).T
    )  # [128, 2]
    in_maps = []
    for i in range(N_CORES):
        in_maps.append(
            {
                "x": x[i].astype(bf16),
                "wqkvT": wqkvT,
                "woutT": woutT,
                "bias": bias,
            }
        )
    return in_maps


def kernel(x, w_qkv, w_out, b_out, _trace=False):
    nc = _get_nc()
    in_maps = make_in_maps(x, w_qkv, w_out, b_out)
    res = run_bass_kernel_spmd(
        nc, in_maps, core_ids=list(range(N_CORES)), trace=_trace
    )
    out = np.stack([res.results[i]["out"] for i in range(N_CORES)], axis=0)
    if _trace:
        kernel.last_exec_time_ns = res.exec_time_ns
        kernel.last_results = res
    return out
